# revision 77
# baseline (speedup 1.0000x reference)
"""Trainium2 Bass kernel for AnyGPT local-attention block (8 NeuronCores).

Sharding: (batch, seq-half) -> 8 shards of 1024 query tokens each; every core
gets a 256-token k/v halo (zero-padded at sequence start), so no collectives
are needed and the host gather is a pure concatenation.

Per-core pipeline (LayerNorm/softmax math in fp32):
  q/k projections in bf16 (W^T-major, [H, tok] layout); v and out projections
  in fp8e4 DoubleRow matmuls that contract 256 rows per instruction (kb-pairs
  folded into the pair dim), with power-of-2 scales (w x64, ctx x8) absorbed
  by the v "ones" column (8.0) and 512x-scaled LayerNorm constants so no
  extra rescale ops exist: the residual is added into the out-proj PSUM chain
  via an identity matmul and the final evict multiplies by 1/512. Banded
  scores are computed TRANSPOSED ([key, query]) in 128-query x 3x128-key
  blocks (the middle key block needs no mask) so exp'd probs feed the ctx
  matmul directly; softmax is unnormalized with denominators taken from the
  ones row, staged once per query block and bulk-reciprocated.

Softmax denominators ride partition 64 of the ctx PSUM (the vones row), so
ONE DVE cast per item evicts ctx+denoms together into a par-split ct layout;
per query block the denom row is DMA-gathered to [head, q], reciprocated,
bounced through a DRAM scratch and spread back per-head with
partition-broadcast DMAs so the normalize is a plain bf16 DVE multiply.

Scheduling: a single pool region overlaps everything. The pre-phase runs q/k
projections for the first query blocks, all v-projection (DoubleRow) and the
LayerNorm stats + ONE batched ACT Sqrt (so the activation table switches
sqrt<->exp exactly once); the per-block LN normalizes (DVE tensor_scalar --
NEVER GPSIMD, whose 2-scalar path costs ~13us/call) drip into the first
items. Blocks run [2,3,1,0,4,5,6,7] so the halo-key blocks come early and
the remaining k/q projection chunks drip one PSUM group per item with their
ACT evictions deferred one item (PSUM dep lands before the copy heads the
in-order ACT queue). Each query block's finish is staged (den-gather DMA at
the block boundary, reciprocal two items later) and its normalize/out-proj
tasks interleave with the next block's attention, out-proj evictions
trailing their matmuls by two task slots. Engine placement: ACT = exp +
projection/output evictions, DVE = ctx casts + LN + normalize, GPSIMD =
band masks only (it cannot read PSUM).

Measured (min of 4): ~226us; rel_err 1.14e-2. Beware ~+-3% run-to-run
noise when comparing scheduling variants -- measure min-of-N.
"""

import numpy as np
import ml_dtypes

import concourse.bass as bass
import concourse.mybir as mybir
import concourse.tile as tile
from concourse import bacc

F32 = mybir.dt.float32
BF16 = mybir.dt.bfloat16
F8 = mybir.dt.float8e4

B, S, H, NH, HD, WIN = 4, 2048, 1024, 16, 64, 256
P = 128
SQ = 1024          # queries per core
SE = SQ + WIN      # ext tokens (halo + queries)
KB = H // P        # 8 contraction blocks
KB2 = KB // 2      # 4 DoubleRow kb-pairs
QBS = 256          # query block size in attention
NQB = SQ // QBS    # 4
JBN = 4            # key blocks of 128 per query block
LN_EPS = 1e-7
NCORES = 8
WSC = 64.0         # fp8 weight scale
CSC = 8.0          # fp8 ctx scale (vones = WSC / CSC)
OSC = WSC * CSC    # 512: out-proj PSUM scale = LN-const scale

AF = mybir.ActivationFunctionType
ALU = mybir.AluOpType
DR = mybir.MatmulPerfMode.DoubleRow


def _bcast_ap(handle, n_part):
    """[D] DRAM vector -> [n_part, D] partition-broadcast AP (step 0)."""
    ap = handle[:]
    return bass.AP(tensor=ap.tensor, offset=ap.offset, ap=[[0, n_part]] + list(ap.ap))


def build_nc():
    nc = bacc.Bacc("TRN2", target_bir_lowering=False, debug=False)

    xq_h = nc.declare_dram_parameter("xq", [SQ, H], BF16, isOutput=False)
    xT_h = nc.declare_dram_parameter("xT", [H, SE], BF16, isOutput=False)
    xT8_h = nc.declare_dram_parameter("xT8", [H, SE], F8, isOutput=False)
    wqT_h = nc.declare_dram_parameter("wqT", [H, H], BF16, isOutput=False)
    wkT_h = nc.declare_dram_parameter("wkT", [H, H], BF16, isOutput=False)
    wv8_h = nc.declare_dram_parameter("wv8", [H, H], F8, isOutput=False)
    wo8_h = nc.declare_dram_parameter("wo8", [H, H], F8, isOutput=False)
    lnw_h = nc.declare_dram_parameter("lnw", [H], BF16, isOutput=False)
    lnbbo_h = nc.declare_dram_parameter("lnbbo", [H], BF16, isOutput=False)
    vones_h = nc.declare_dram_parameter("vones", [SE], BF16, isOutput=False)
    ident_h = nc.declare_dram_parameter("ident", [P, P], BF16, isOutput=False)
    out_h = nc.declare_dram_parameter("out", [SQ, H], F32, isOutput=True)
    # DRAM bounce buffer for the per-head reciprocal spread (SBUF APs cannot
    # partition-broadcast, DRAM APs can)
    rs_h = nc.declare_dram_parameter("rs", [NQB, NH, QBS], BF16,
                                     isOutput=True)

    with tile.TileContext(nc) as tc:
        _body(tc, nc, xq_h, xT_h, xT8_h, wqT_h, wkT_h, wv8_h, wo8_h,
              lnw_h, lnbbo_h, vones_h, ident_h, out_h, rs_h)
    nc.compile()
    return nc


def _body(tc, nc, xq_h, xT_h, xT8_h, wqT_h, wkT_h, wv8_h, wo8_h,
          lnw_h, lnbbo_h, vones_h, ident_h, out_h, rs_h):
    with (
        tc.tile_pool(name="const", bufs=1) as const,
        tc.tile_pool(name="big", bufs=1) as big,
        tc.tile_pool(name="wpool", bufs=16) as wpool,
        tc.tile_pool(name="w8pool", bufs=8) as w8pool,
        tc.tile_pool(name="work", bufs=3) as work,
        tc.tile_pool(name="lnpool", bufs=2) as lnpool,
    ):
        # ---- constants ----
        lnw_b = const.tile([P, H], BF16)
        nc.sync.dma_start(lnw_b[:], _bcast_ap(lnw_h, P))
        lnbbo_b = const.tile([P, H], BF16)
        nc.sync.dma_start(lnbbo_b[:], _bcast_ap(lnbbo_h, P))
        eps_t = const.tile([P, 1], F32)
        nc.vector.memset(eps_t[:], LN_EPS)
        ident_sb = const.tile([P, P], BF16)
        nc.sync.dma_start(ident_sb[:], ident_h[:])

        # ---- x^T resident [128, kb, tok]; interleave each kb's xt chunk with
        # its wq slice so the first q-proj matmul's operands land earliest ----
        xt_sb = big.tile([P, KB, SE], BF16, tag="xtr")
        xt8_sb = big.tile([P, KB, SE], F8, tag="xtr8", name="xt8")
        wq_sl = [wpool.tile([P, H], BF16, tag="wslice", name=f"wq_{kb}")
                 for kb in range(KB)]
        wv8_sl = [w8pool.tile([P, 2, H], F8, tag="w8", name=f"wv8_{j}")
                  for j in range(KB2)]
        for kb in range(KB):
            nc.sync.dma_start(xt_sb[:, kb, WIN:WIN + 512],
                              xT_h[:][kb * P:(kb + 1) * P, WIN:WIN + 512])
            nc.sync.dma_start(wq_sl[kb][:], wqT_h[:][kb * P:(kb + 1) * P, :])

        qT_sb = big.tile([P, KB, SQ], BF16)    # q^T  [H, 1024]
        kT_sb = big.tile([P, KB, SE], BF16)    # k^T  [H, 1280]
        # v natural + a "ones" column (value 8 = WSC/CSC; 0 for zero-padded
        # halo tokens) so pad keys contribute nothing and the fp8 scales
        # cancel through the softmax normalization
        v_sb = big.tile([P, SE // P, NH, HD + 1], BF16)
        vo = vones_h[:]
        vo_pt = bass.AP(tensor=vo.tensor, offset=vo.offset,
                        ap=[[1, P], [P, SE // P]])
        for h in range(NH):
            nc.sync.dma_start(v_sb[:, :, h, HD], vo_pt)
        # UNnormalized ctx^T in par-split layout [dim(65), par, hb, q]:
        # partition 64 carries the softmax denominators (the vones row of the
        # ctx matmul) so ONE CAST per item evicts ctx and denominators
        ct_sb = big.tile([HD + 1, 2, KB, SQ], BF16)
        recip_sb = big.tile([NH, SQ], BF16)    # 1/den (den staged, then recip)
        res_sb = big.tile([P, KB, H], BF16)    # 512 * LayerNorm residual

        # everything below shares one pool region so projection groups can
        # interleave with attention items on the in-order engine queues
        merged_pools = (
            tc.tile_pool(name="gpsum", bufs=2, space="PSUM"),
            tc.tile_pool(name="spsum", bufs=2, space="PSUM"),
            tc.tile_pool(name="cpsum", bufs=2, space="PSUM"),
        )
        with merged_pools[0] as gpsum, merged_pools[1] as spsum, \
                merged_pools[2] as cpsum:
            # wk slices requested right behind wq/xt so the DMA queues have
            # them in flight well before the k-projection starts
            wk_sl = [wpool.tile([P, H], BF16, tag="wslice", name=f"wk_{kb}")
                     for kb in range(KB)]
            for kb in range(KB):
                nc.sync.dma_start(wk_sl[kb][:],
                                  wkT_h[:][kb * P:(kb + 1) * P, :])
            for kb in range(KB):
                nc.sync.dma_start(xt8_sb[:, kb, :],
                                  xT8_h[:][kb * P:(kb + 1) * P, :])
            # w8 slice [p, i, n] = 64*w[n, (2*kbp+i)*128+p] via strided DMA
            wo8_sl = [w8pool.tile([P, 2, H], F8, tag="w8", name=f"wo8_{j}")
                      for j in range(KB2)]
            for j in range(KB2):
                src = wv8_h[:]
                ap = bass.AP(tensor=src.tensor,
                             offset=src.offset + 2 * j * P * H,
                             ap=[[H, P], [P * H, 2], [1, H]])
                nc.sync.dma_start(wv8_sl[j][:], ap)
            # the x^T halo columns feed the k-halo chunks needed by blocks
            # q8=1,0 early in the item order, so they stream before the tail
            for kb in range(KB):
                nc.sync.dma_start(xt_sb[:, kb, :WIN],
                                  xT_h[:][kb * P:(kb + 1) * P, :WIN])
                nc.sync.dma_start(xt_sb[:, kb, WIN + 512:],
                                  xT_h[:][kb * P:(kb + 1) * P, WIN + 512:])
            for j in range(KB2):
                src = wo8_h[:]
                ap = bass.AP(tensor=src.tensor,
                             offset=src.offset + 2 * j * P * H,
                             ap=[[H, P], [P * H, 2], [1, H]])
                nc.sync.dma_start(wo8_sl[j][:], ap)

            # ---- projection group emitters (one PSUM group each) ----
            def qk_mms(dst, wsl, ob, tok0, i0, ilen):
                ps = gpsum.tile([P, 512], F32, tag="gp", name="ps_g")
                for kb in range(KB):
                    nc.tensor.matmul(
                        ps[:, :ilen],
                        wsl[kb][:, ob * P:(ob + 1) * P],
                        xt_sb[:, kb, tok0 + i0: tok0 + i0 + ilen],
                        start=(kb == 0), stop=(kb == KB - 1),
                    )
                # the eviction runs one item later (see the bg drip) so the
                # ACT queue never stalls waiting for this group's PSUM
                return lambda: nc.scalar.copy(out=dst[:, ob, i0:i0 + ilen],
                                              in_=ps[:, :ilen])

            def qk_group(dst, wsl, ob, tok0, i0, ilen):
                qk_mms(dst, wsl, ob, tok0, i0, ilen)()

            def v_group(tt, oh):
                # v = x @ (64 wv)^T in fp8 DoubleRow, contracting 256 rows
                # (one kb-pair) per matmul; evict on DVE to keep the ACT
                # queue short in the pre-phase
                ps = gpsum.tile([P, 512], F32, tag="gp", name="ps_g")
                for j in range(KB2):
                    nc.tensor.matmul(
                        ps[:],
                        xt8_sb[:, 2 * j:2 * j + 2, tt * P:(tt + 1) * P],
                        wv8_sl[j][:, :, oh * 512:(oh + 1) * 512],
                        start=(j == 0), stop=(j == KB2 - 1),
                        perf_mode=DR,
                    )
                nc.vector.tensor_copy(
                    out=v_sb[:, tt, oh * 8:(oh + 1) * 8, 0:HD],
                    in_=ps[:].rearrange("p (h d) -> p h d", d=HD),
                )

            # ---- LayerNorm residual; 512x-scaled constants make res_sb =
            # 512*residual so the final evict's 1/512 recovers it exactly.
            # Elementwise work on DVE (GPSIMD's 2-scalar tensor_scalar is a
            # ~13us software path that serialized the whole kernel); x stages
            # in-place in res_sb. Stats for all 8 blocks batch into ONE ACT
            # Sqrt so the activation table switches sqrt<->exp only once. ----
            mvall = lnpool.tile([P, KB, 2], F32, tag="mvall", name="mvall",
                                bufs=1)
            rstdall = lnpool.tile([P, 2, KB], F32, tag="rstdall",
                                  name="rstdall", bufs=1)

            def ln_stats(it):
                x_t = res_sb[:, it, :]
                nc.sync.dma_start(x_t, xq_h[:][it * P:(it + 1) * P, :])
                stats = lnpool.tile([P, 2, 6], F32, tag="stats", name="stats")
                for g in range(2):
                    nc.vector.bn_stats(out=stats[:, g, :],
                                       in_=x_t[:, g * 512:(g + 1) * 512])
                nc.vector.bn_aggr(out=mvall[:, it, :], in_=stats[:])

            def ln_rstd():
                # rstdall row 0 = 1/std, row 1 = mu/std (all 8 blocks at once)
                nc.scalar.activation(out=rstdall[:, 0, :],
                                     in_=mvall[:, :, 1], func=AF.Sqrt,
                                     bias=eps_t[:])
                nc.vector.reciprocal(out=rstdall[:, 0, :],
                                     in_=rstdall[:, 0, :])
                nc.vector.tensor_mul(out=rstdall[:, 1, :],
                                     in0=mvall[:, :, 0],
                                     in1=rstdall[:, 0, :])

            def ln_norm(it):
                x_t = res_sb[:, it, :]
                nc.vector.tensor_scalar(out=x_t, in0=x_t,
                                        scalar1=rstdall[:, 0, it:it + 1],
                                        scalar2=rstdall[:, 1, it:it + 1],
                                        op0=ALU.mult, op1=ALU.subtract)
                nc.vector.tensor_mul(out=x_t, in0=x_t, in1=lnw_b[:])
                nc.vector.tensor_add(out=x_t, in0=x_t, in1=lnbbo_b[:])

            # ---- attention: scores^T -> exp -> mask -> ctx^T -> recips ----
            # 128-query blocks x 3 key blocks of 128 (the middle key block is
            # fully inside the band and needs no mask), so exp/mask/matmul
            # volume is 3/4 of the 256-query variant. Head-PAIR iterations:
            # the even head's score matmuls contract on PE rows 0-63, the odd
            # head's on rows 64-127 (tile_position auto-derived from the lhsT
            # base partition) so the hardware runs each jb's pair
            # concurrently. Software-pipelined with a 2-iteration lookahead
            # so the in-order PE never waits on exp/mask.
            NQ8 = SQ // P      # 8 query blocks
            JB3 = 3            # key blocks per query block
            # q8 = 1, 0 go LAST: they are the only blocks reading the x^T
            # halo columns (keys < 256), whose k-projection chunk streams in
            # the background; pairs stay qb-adjacent so finish_qb still
            # fires every 16 items
            # halo-consuming blocks 1, 0 run EARLY (their k-halo chunk heads
            # the bg queue) so all other bg deadlines relax and the bg work
            # spreads deep into the item stream instead of ending at item ~31
            q8_order = [2, 3, 1, 0, 4, 5, 6, 7]
            items = [(q8, hb) for q8 in q8_order for hb in range(NH // 2)]
            probs_of = {}

            def emit_scores(i):
                q8, hb = items[i]
                probs = work.tile([P, 2, JB3, P], BF16, tag="probs",
                                  name="probs", bufs=3)
                ps_s = spsum.tile([P, 2, 4, P], F32, tag="sc", name="ps_s")
                for jb in range(JB3):
                    j0 = (q8 + jb) * P
                    for par in range(2):
                        ho = par * HD
                        nc.tensor.matmul(
                            ps_s[:, par, jb, :],
                            kT_sb[ho:ho + HD, hb, j0:j0 + P],
                            qT_sb[ho:ho + HD, hb, q8 * P:(q8 + 1) * P],
                            start=True, stop=True,
                        )
                nc.scalar.activation(out=probs[:],
                                     in_=ps_s[:, :, 0:JB3, :], func=AF.Exp)
                # band mask: one affine inequality over (key row r, query
                # col c) per outer key block, on the Pool engine. r =
                # partition, parity is a dead dim (step 0), c is the last
                # free dim. Keep where A >= 0, zero elsewhere:
                #   jb0: r-c-1>=0   jb1: always in band   jb2: c-r>=0
                for jb, (ch, cstep, base) in ((0, (1, -1, -1)),
                                              (2, (-1, 1, 0))):
                    nc.gpsimd.affine_select(
                        out=probs[:, :, jb, :], in_=probs[:, :, jb, :],
                        compare_op=ALU.is_ge, fill=0.0, base=base,
                        pattern=[[0, 2], [cstep, P]],
                        channel_multiplier=ch)
                probs_of[i] = probs

            def emit_ctx(i):
                q8, hb = items[i]
                probs = probs_of.pop(i)
                ps_c = cpsum.tile([HD + 1, 2, P], F32, tag="cx", name="ps_c")
                for par in range(2):
                    for jb in range(JB3):
                        nc.tensor.matmul(
                            ps_c[:, par, :],
                            v_sb[:, q8 + jb, 2 * hb + par, :],
                            probs[:, par, jb, :],
                            start=(jb == 0), stop=(jb == JB3 - 1),
                        )
                qs = slice(q8 * P, (q8 + 1) * P)
                # one CAST evicts both heads' ctx AND the denominator row
                with nc.allow_low_precision(
                        reason="softmax denom in bf16: 0.4% rel "
                               "on a 2e-2 budget"):
                    nc.vector.tensor_copy(out=ct_sb[:, :, hb, qs],
                                          in_=ps_c[:, :, :])

            def finish_a(qlo, qlen):
                # stage 1: gather the denominator row (ct partition 64) to
                # [head, q] rows with one DMA (pure DMA -- no engine waits)
                qs = slice(qlo, qlo + qlen)
                # recip_sb rows are par-major: head h = 2*hb+par at row
                # par*8+hb, so [par, hb] merges into one stride-1024 dim
                den = ct_sb[HD:HD + 1, :, :, qs]
                nc.sync.dma_start(
                    recip_sb[:, qs],
                    bass.AP(tensor=den.tensor, offset=den.offset,
                            ap=[list(den.ap)[0]] +
                               [[KB * SQ, 2], [SQ, KB], [1, qlen]]))

            def finish_b(qlo, qlen):
                # stage 2 (two items later, so the gather DMA has landed and
                # the reciprocal does not stall the DVE queue): reciprocate,
                # bounce through DRAM, and spread each head's row across its
                # 64 ct partitions so the normalize is a plain DVE mult
                qs = slice(qlo, qlo + qlen)
                qb, off = divmod(qlo, QBS)
                with nc.allow_low_precision(
                        reason="softmax denom recip in bf16: 0.4% rel "
                               "on a 2e-2 budget"):
                    nc.vector.reciprocal(out=recip_sb[:, qs],
                                         in_=recip_sb[:, qs])
                nc.sync.dma_start(rs_h[:][qb, :, off:off + qlen],
                                  recip_sb[:, qs])
                rcp2 = work.tile([HD, 2, KB, qlen], BF16, tag="rcp2",
                                 name="rcp2", bufs=1)
                for h in range(NH):
                    src = rs_h[:][qb, (h % 2) * KB + h // 2, off:off + qlen]
                    nc.sync.dma_start(
                        rcp2[:, h % 2, h // 2, :],
                        bass.AP(tensor=src.tensor, offset=src.offset,
                                ap=[[0, HD]] + list(src.ap)))
                rcp2_of[qlo] = rcp2

            # ---- final tasks: normalize ctx^T into fp8 (x8 scale via
            # vones=8) and out-project in fp8 DoubleRow + residual via
            # identity matmul. Both are sliced per 128-query block, so block
            # q8's 10 tasks interleave with block q8+1's attention pairs
            # (PE/Pool have slack there while ACT runs exp); only q8=7's
            # tasks trail the last pair. ----
            ct8 = big.tile([P, KB, SQ], F8, tag="xtr8", name="ct8")

            def emit_norm(qlo, qlen, hb, par):
                # normalize ct by the DMA-spread reciprocals: plain SBUF
                # bf16 DVE multiply (no PE selector matmul needed); par halves
                # relocate [0:64] -> [par*64:...] like the old 2-CAST evict
                qs = slice(qlo, qlo + qlen)
                nc.vector.tensor_mul(out=ct8[par * HD:(par + 1) * HD, hb, qs],
                                     in0=ct_sb[0:HD, par, hb, qs],
                                     in1=rcp2_of[qlo][:, par, hb, :])

            def emit_out_mms(it, oh):
                ps_o = gpsum.tile([P, 512], F32, tag="gp", name="ps_o")
                for j in range(KB2):
                    nc.tensor.matmul(
                        ps_o[:],
                        ct8[:, 2 * j:2 * j + 2, it * P:(it + 1) * P],
                        wo8_sl[j][:, :, oh * 512:(oh + 1) * 512],
                        start=(j == 0), stop=False,
                        perf_mode=DR,
                    )
                nc.tensor.matmul(
                    ps_o[:], ident_sb[:],
                    res_sb[:, it, oh * 512:(oh + 1) * 512],
                    start=False, stop=True,
                )

                def evict():
                    o_t = work.tile([P, 512], F32, tag="o_t", name="o_t",
                                    bufs=2)
                    nc.scalar.mul(o_t[:], ps_o[:], 1.0 / OSC)
                    nc.sync.dma_start(
                        out_h[:][it * P:(it + 1) * P,
                                 oh * 512:(oh + 1) * 512],
                        o_t[:])
                return evict

            def final_tasks(qlo, qlen):
                for hb in range(KB):
                    yield lambda hb=hb: (emit_norm(qlo, qlen, hb, 0),
                                         emit_norm(qlo, qlen, hb, 1))
                # out-proj evictions trail their matmuls by two task slots so
                # the ACT-queue copy never waits on an unfinished PSUM group
                ev = []
                for it in range(qlo // P, (qlo + qlen) // P):
                    for oh in range(2):
                        def mm(it=it, oh=oh):
                            ev.append(emit_out_mms(it, oh))
                        yield mm
                        if len(ev) >= (2 if oh == 1 else 3):
                            yield ev.pop(0)
                while ev:
                    yield ev.pop(0)

            rcp2_of = {}

            def pe_warm(n):
                # dummy back-to-back matmuls to keep/get the PE HAM clock
                # gate at 8/8 across a known idle window (cold MMs run at
                # 1.2 instead of 2.4 GHz); they cost nothing when the PE
                # would otherwise idle
                ps_w = gpsum.tile([P, 512], F32, tag="gp", name="ps_w")
                for _ in range(n):
                    nc.tensor.matmul(ps_w[:, 0:P], ident_sb[:], ident_sb[:],
                                     start=True, stop=True)

            # ---- pre-phase: just enough projection work for the first
            # attention items (q chunk0, k chunk0), then all v groups (they
            # are DoubleRow-cheap, and xt8 -- whose SBUF ring slot ct8
            # reuses -- must be fully consumed before the first normalize
            # task writes ct8), then LayerNorm. PE-warming dummies run while
            # the first groups' weights/activations stream in. ----
            for ob in range(KB):
                qk_group(qT_sb, wq_sl, ob, WIN, 0, 512)
            for ob in range(KB):
                qk_group(kT_sb, wk_sl, ob, 0, 256, 512)
            for tt in range(SE // P):
                v_group(tt, 0)
                v_group(tt, 1)
            # LayerNorm last in the pre-phase; all stats first, then the one
            # batched Sqrt. The 8 normalize units drip between the first
            # attention items (below) so their ~14us of DVE elementwise work
            # does not delay the first ctx evictions.
            for it in range(KB):
                ln_stats(it)
            ln_rstd()

            # ---- remaining projection groups drip-fed between attention
            # items, ordered by deadline (k halo for blocks 1,0 first, then
            # the k/q chunks for blocks 4-7, ob-interleaved to match the
            # per-head consumption order) ----
            def bgwork():
                for ob in range(KB):
                    yield lambda ob=ob: qk_mms(kT_sb, wk_sl, ob, 0, 0, 256)
                for ob in range(KB):
                    yield lambda ob=ob: qk_mms(kT_sb, wk_sl, ob, 0, 768, 512)
                for ob in range(KB):
                    yield lambda ob=ob: qk_mms(qT_sb, wq_sl, ob, WIN, 512,
                                               512)

            bg = bgwork()
            pending = []
            fb_at = {}
            bg_evict = None
            delay = 0
            emit_scores(0)
            emit_scores(1)
            for i in range(len(items)):
                if i + 2 < len(items):
                    emit_scores(i + 2)
                emit_ctx(i)
                if i < KB:
                    ln_norm(i)
                # halo chunks must be emitted by item ~14 (blocks 1,0 start
                # at 16) and each q512(ob) before the block-4 scores that
                # read it (emitted at item 30+ob). Each group's eviction is
                # deferred to the NEXT item so its PSUM dependency lands
                # before the copy reaches the head of the ACT queue.
                if bg_evict is not None:
                    bg_evict()
                    bg_evict = None
                if i < 16 or i % 2 == 0:
                    bgtask = next(bg, None)
                    if bgtask is not None:
                        bg_evict = bgtask()
                if (i + 1) % NH == 0:
                    fin = ((items[i][0] // 2) * QBS, QBS)
                    finish_a(*fin)
                    fb_at[i + 2] = fin
                if i in fb_at:
                    fin = fb_at.pop(i)
                    finish_b(*fin)
                    pending.append(final_tasks(*fin))
                    # let the spread-DMA latency pass before the first norm
                    # task queues on DVE
                    delay = 2
                if delay > 0:
                    delay -= 1
                else:
                    # 10 tasks per 8-item block -> 2 tasks/item keeps the
                    # out-proj right behind attention
                    for _ in range(2):
                        while pending:
                            task = next(pending[0], None)
                            if task is not None:
                                task()
                                break
                            pending.pop(0)
            if bg_evict is not None:
                bg_evict()
            for fin in fb_at.values():
                finish_b(*fin)
                pending.append(final_tasks(*fin))
            # the finish chain of the last block leaves the PE idle in ~1-2us
            # bursts (recip/spread DMA latency + DVE norms) that sum past the
            # HAM re-throttle window; interleave dummy-matmul batches between
            # the tail tasks so the final out-proj matmuls run at full clock
            pe_warm(40)
            tail_i = 0
            for gen in pending:
                for task in gen:
                    task()
                    # only during the DVE-norm drain; dummies after the last
                    # out-proj matmuls would extend the kernel instead
                    if tail_i < KB:
                        pe_warm(16)
                    tail_i += 1


_CACHE = {}


def get_nc():
    if "nc" not in _CACHE:
        _CACHE["nc"] = build_nc()
    return _CACHE["nc"]


def make_in_maps(inputs):
    x = np.asarray(inputs["hidden_states"], dtype=np.float32)
    wq = np.asarray(inputs["wq"], dtype=np.float32)
    wk = np.asarray(inputs["wk"], dtype=np.float32)
    wv = np.asarray(inputs["wv"], dtype=np.float32)
    wo = np.asarray(inputs["wo"], dtype=np.float32)
    bo = np.asarray(inputs["bo"], dtype=np.float32)
    ln_w = np.asarray(inputs["ln_w"], dtype=np.float32)
    ln_b = np.asarray(inputs["ln_b"], dtype=np.float32)

    bf = ml_dtypes.bfloat16
    f8 = ml_dtypes.float8_e4m3
    wqT = np.ascontiguousarray(wq.T).astype(bf)
    wkT = np.ascontiguousarray(wk.T).astype(bf)
    wv8 = np.clip(np.ascontiguousarray(wv.T) * WSC, -240, 240).astype(f8)
    wo8 = np.clip(np.ascontiguousarray(wo.T) * WSC, -240, 240).astype(f8)
    lnw = (ln_w * OSC).astype(bf)
    lnbbo = ((ln_b + bo) * OSC).astype(bf)
    ident = np.eye(P, dtype=np.float32).astype(bf)

    in_maps = []
    for core in range(NCORES):
        b, hh = divmod(core, 2)
        start = hh * SQ
        xkv = np.zeros((SE, H), dtype=np.float32)
        xkv[WIN:] = x[b, start:start + SQ]
        vones = np.full(SE, WSC / CSC, dtype=np.float32)
        if start > 0:
            xkv[:WIN] = x[b, start - WIN:start]
        else:
            vones[:WIN] = 0.0
        xkvT = np.ascontiguousarray(xkv.T)
        in_maps.append({
            "xq": np.ascontiguousarray(x[b, start:start + SQ]).astype(bf),
            "xT": xkvT.astype(bf),
            "xT8": np.clip(xkvT, -240, 240).astype(f8),
            "wqT": wqT, "wkT": wkT, "wv8": wv8, "wo8": wo8,
            "lnw": lnw, "lnbbo": lnbbo,
            "vones": vones.astype(bf),
            "ident": ident,
        })
    return in_maps


def kernel(**inputs):
    from concourse.bass_utils import run_bass_kernel_spmd
    nc = get_nc()
    in_maps = make_in_maps(inputs)
    res = run_bass_kernel_spmd(nc, in_maps, core_ids=list(range(NCORES)))
    out = np.empty((B, S, H), dtype=np.float32)
    for core in range(NCORES):
        b, hh = divmod(core, 2)
        out[b, hh * SQ:(hh + 1) * SQ, :] = res.results[core]["out"]
    return out



# revision 78
# speedup vs baseline: 1.0045x; 1.0045x over previous
"""Trainium2 Bass kernel for AnyGPT local-attention block (8 NeuronCores).

Sharding: (batch, seq-half) -> 8 shards of 1024 query tokens each; every core
gets a 256-token k/v halo (zero-padded at sequence start), so no collectives
are needed and the host gather is a pure concatenation.

Per-core pipeline (LayerNorm/softmax math in fp32):
  q/k projections in bf16 (W^T-major, [H, tok] layout); v and out projections
  in fp8e4 DoubleRow matmuls that contract 256 rows per instruction (kb-pairs
  folded into the pair dim), with power-of-2 scales (w x64, ctx x8) absorbed
  by the v "ones" column (8.0) and 512x-scaled LayerNorm constants so no
  extra rescale ops exist: the residual is added into the out-proj PSUM chain
  via an identity matmul and the final evict multiplies by 1/512. Banded
  scores are computed TRANSPOSED ([key, query]) in 128-query x 3x128-key
  blocks (the middle key block needs no mask) so exp'd probs feed the ctx
  matmul directly; softmax is unnormalized with denominators taken from the
  ones row, staged once per query block and bulk-reciprocated.

Softmax denominators ride partition 64 of the ctx PSUM (the vones row), so
ONE DVE cast per item evicts ctx+denoms together into a par-split ct layout;
per query block the denom row is DMA-gathered to [head, q], reciprocated,
bounced through a DRAM scratch and spread back per-head with
partition-broadcast DMAs so the normalize is a plain bf16 DVE multiply.

Scheduling: a single pool region overlaps everything. The pre-phase runs q/k
projections for the first query blocks, all v-projection (DoubleRow) and the
LayerNorm stats + ONE batched ACT Sqrt (so the activation table switches
sqrt<->exp exactly once); the per-block LN normalizes (DVE tensor_scalar --
NEVER GPSIMD, whose 2-scalar path costs ~13us/call) drip into the first
items. Blocks run [2,3,1,0,4,5,6,7] so the halo-key blocks come early and
the remaining k/q projection chunks drip one PSUM group per item with their
ACT evictions deferred one item (PSUM dep lands before the copy heads the
in-order ACT queue). Each query block's finish is staged (den-gather DMA at
the block boundary, reciprocal two items later) and its normalize/out-proj
tasks interleave with the next block's attention, out-proj evictions
trailing their matmuls by two task slots. Engine placement: ACT = exp +
projection/output evictions, DVE = ctx casts + LN + normalize, GPSIMD =
band masks only (it cannot read PSUM).

Measured (min of 4): ~226us; rel_err 1.14e-2. Beware ~+-3% run-to-run
noise when comparing scheduling variants -- measure min-of-N.
"""

import numpy as np
import ml_dtypes

import concourse.bass as bass
import concourse.mybir as mybir
import concourse.tile as tile
from concourse import bacc

F32 = mybir.dt.float32
BF16 = mybir.dt.bfloat16
F8 = mybir.dt.float8e4

B, S, H, NH, HD, WIN = 4, 2048, 1024, 16, 64, 256
P = 128
SQ = 1024          # queries per core
SE = SQ + WIN      # ext tokens (halo + queries)
KB = H // P        # 8 contraction blocks
KB2 = KB // 2      # 4 DoubleRow kb-pairs
QBS = 256          # query block size in attention
NQB = SQ // QBS    # 4
JBN = 4            # key blocks of 128 per query block
LN_EPS = 1e-7
NCORES = 8
WSC = 64.0         # fp8 weight scale
CSC = 8.0          # fp8 ctx scale (vones = WSC / CSC)
OSC = WSC * CSC    # 512: out-proj PSUM scale = LN-const scale

AF = mybir.ActivationFunctionType
ALU = mybir.AluOpType
DR = mybir.MatmulPerfMode.DoubleRow


def _bcast_ap(handle, n_part):
    """[D] DRAM vector -> [n_part, D] partition-broadcast AP (step 0)."""
    ap = handle[:]
    return bass.AP(tensor=ap.tensor, offset=ap.offset, ap=[[0, n_part]] + list(ap.ap))


def build_nc():
    nc = bacc.Bacc("TRN2", target_bir_lowering=False, debug=False)

    xq_h = nc.declare_dram_parameter("xq", [SQ, H], BF16, isOutput=False)
    xT_h = nc.declare_dram_parameter("xT", [H, SE], BF16, isOutput=False)
    xT8_h = nc.declare_dram_parameter("xT8", [H, SE], F8, isOutput=False)
    wqT_h = nc.declare_dram_parameter("wqT", [H, H], BF16, isOutput=False)
    wkT_h = nc.declare_dram_parameter("wkT", [H, H], BF16, isOutput=False)
    wv8_h = nc.declare_dram_parameter("wv8", [H, H], F8, isOutput=False)
    wo8_h = nc.declare_dram_parameter("wo8", [H, H], F8, isOutput=False)
    lnw_h = nc.declare_dram_parameter("lnw", [H], BF16, isOutput=False)
    lnbbo_h = nc.declare_dram_parameter("lnbbo", [H], BF16, isOutput=False)
    vones_h = nc.declare_dram_parameter("vones", [SE], BF16, isOutput=False)
    ident_h = nc.declare_dram_parameter("ident", [P, P], BF16, isOutput=False)
    out_h = nc.declare_dram_parameter("out", [SQ, H], F32, isOutput=True)
    # DRAM bounce buffer for the per-head reciprocal spread (SBUF APs cannot
    # partition-broadcast, DRAM APs can)
    rs_h = nc.declare_dram_parameter("rs", [NQB, NH, QBS], BF16,
                                     isOutput=True)

    with tile.TileContext(nc) as tc:
        _body(tc, nc, xq_h, xT_h, xT8_h, wqT_h, wkT_h, wv8_h, wo8_h,
              lnw_h, lnbbo_h, vones_h, ident_h, out_h, rs_h)
    nc.compile()
    return nc


def _body(tc, nc, xq_h, xT_h, xT8_h, wqT_h, wkT_h, wv8_h, wo8_h,
          lnw_h, lnbbo_h, vones_h, ident_h, out_h, rs_h):
    with (
        tc.tile_pool(name="const", bufs=1) as const,
        tc.tile_pool(name="big", bufs=1) as big,
        tc.tile_pool(name="wpool", bufs=16) as wpool,
        tc.tile_pool(name="w8pool", bufs=8) as w8pool,
        tc.tile_pool(name="work", bufs=3) as work,
        tc.tile_pool(name="lnpool", bufs=2) as lnpool,
    ):
        # ---- constants ----
        lnw_b = const.tile([P, H], BF16)
        nc.sync.dma_start(lnw_b[:], _bcast_ap(lnw_h, P))
        lnbbo_b = const.tile([P, H], BF16)
        nc.sync.dma_start(lnbbo_b[:], _bcast_ap(lnbbo_h, P))
        eps_t = const.tile([P, 1], F32)
        nc.vector.memset(eps_t[:], LN_EPS)
        ident_sb = const.tile([P, P], BF16)
        nc.sync.dma_start(ident_sb[:], ident_h[:])

        # ---- x^T resident [128, kb, tok]; interleave each kb's xt chunk with
        # its wq slice so the first q-proj matmul's operands land earliest ----
        xt_sb = big.tile([P, KB, SE], BF16, tag="xtr")
        xt8_sb = big.tile([P, KB, SE], F8, tag="xtr8", name="xt8")
        wq_sl = [wpool.tile([P, H], BF16, tag="wslice", name=f"wq_{kb}")
                 for kb in range(KB)]
        wv8_sl = [w8pool.tile([P, 2, H], F8, tag="w8", name=f"wv8_{j}")
                  for j in range(KB2)]
        for kb in range(KB):
            nc.sync.dma_start(xt_sb[:, kb, WIN:WIN + 512],
                              xT_h[:][kb * P:(kb + 1) * P, WIN:WIN + 512])
            nc.sync.dma_start(wq_sl[kb][:], wqT_h[:][kb * P:(kb + 1) * P, :])

        qT_sb = big.tile([P, KB, SQ], BF16)    # q^T  [H, 1024]
        kT_sb = big.tile([P, KB, SE], BF16)    # k^T  [H, 1280]
        # v natural + a "ones" column (value 8 = WSC/CSC; 0 for zero-padded
        # halo tokens) so pad keys contribute nothing and the fp8 scales
        # cancel through the softmax normalization
        v_sb = big.tile([P, SE // P, NH, HD + 1], BF16)
        vo = vones_h[:]
        vo_pt = bass.AP(tensor=vo.tensor, offset=vo.offset,
                        ap=[[1, P], [P, SE // P]])
        for h in range(NH):
            nc.sync.dma_start(v_sb[:, :, h, HD], vo_pt)
        # UNnormalized ctx^T in par-split layout [dim(65), par, hb, q]:
        # partition 64 carries the softmax denominators (the vones row of the
        # ctx matmul) so ONE CAST per item evicts ctx and denominators
        ct_sb = big.tile([HD + 1, 2, KB, SQ], BF16)
        recip_sb = big.tile([NH, SQ], BF16)    # 1/den (den staged, then recip)
        res_sb = big.tile([P, KB, H], BF16)    # 512 * LayerNorm residual

        # everything below shares one pool region so projection groups can
        # interleave with attention items on the in-order engine queues
        merged_pools = (
            tc.tile_pool(name="gpsum", bufs=2, space="PSUM"),
            tc.tile_pool(name="spsum", bufs=2, space="PSUM"),
            tc.tile_pool(name="cpsum", bufs=2, space="PSUM"),
        )
        with merged_pools[0] as gpsum, merged_pools[1] as spsum, \
                merged_pools[2] as cpsum:
            # wk slices requested right behind wq/xt so the DMA queues have
            # them in flight well before the k-projection starts
            wk_sl = [wpool.tile([P, H], BF16, tag="wslice", name=f"wk_{kb}")
                     for kb in range(KB)]
            for kb in range(KB):
                nc.sync.dma_start(wk_sl[kb][:],
                                  wkT_h[:][kb * P:(kb + 1) * P, :])
            for kb in range(KB):
                nc.sync.dma_start(xt8_sb[:, kb, :],
                                  xT8_h[:][kb * P:(kb + 1) * P, :])
            # w8 slice [p, i, n] = 64*w[n, (2*kbp+i)*128+p] via strided DMA
            wo8_sl = [w8pool.tile([P, 2, H], F8, tag="w8", name=f"wo8_{j}")
                      for j in range(KB2)]
            for j in range(KB2):
                src = wv8_h[:]
                ap = bass.AP(tensor=src.tensor,
                             offset=src.offset + 2 * j * P * H,
                             ap=[[H, P], [P * H, 2], [1, H]])
                nc.sync.dma_start(wv8_sl[j][:], ap)
            # the x^T halo columns feed the k-halo chunks needed by blocks
            # q8=1,0 early in the item order, so they stream before the tail
            for kb in range(KB):
                nc.sync.dma_start(xt_sb[:, kb, :WIN],
                                  xT_h[:][kb * P:(kb + 1) * P, :WIN])
                nc.sync.dma_start(xt_sb[:, kb, WIN + 512:],
                                  xT_h[:][kb * P:(kb + 1) * P, WIN + 512:])
            for j in range(KB2):
                src = wo8_h[:]
                ap = bass.AP(tensor=src.tensor,
                             offset=src.offset + 2 * j * P * H,
                             ap=[[H, P], [P * H, 2], [1, H]])
                nc.sync.dma_start(wo8_sl[j][:], ap)

            # ---- projection group emitters (one PSUM group each) ----
            def qk_mms(dst, wsl, ob, tok0, i0, ilen):
                ps = gpsum.tile([P, 512], F32, tag="gp", name="ps_g")
                for kb in range(KB):
                    nc.tensor.matmul(
                        ps[:, :ilen],
                        wsl[kb][:, ob * P:(ob + 1) * P],
                        xt_sb[:, kb, tok0 + i0: tok0 + i0 + ilen],
                        start=(kb == 0), stop=(kb == KB - 1),
                    )
                # the eviction runs one item later (see the bg drip) so the
                # ACT queue never stalls waiting for this group's PSUM
                return lambda: nc.scalar.copy(out=dst[:, ob, i0:i0 + ilen],
                                              in_=ps[:, :ilen])

            def qk_group(dst, wsl, ob, tok0, i0, ilen):
                qk_mms(dst, wsl, ob, tok0, i0, ilen)()

            def v_group(tt, oh):
                # v = x @ (64 wv)^T in fp8 DoubleRow, contracting 256 rows
                # (one kb-pair) per matmul; evict on DVE to keep the ACT
                # queue short in the pre-phase
                ps = gpsum.tile([P, 512], F32, tag="gp", name="ps_g")
                for j in range(KB2):
                    nc.tensor.matmul(
                        ps[:],
                        xt8_sb[:, 2 * j:2 * j + 2, tt * P:(tt + 1) * P],
                        wv8_sl[j][:, :, oh * 512:(oh + 1) * 512],
                        start=(j == 0), stop=(j == KB2 - 1),
                        perf_mode=DR,
                    )
                nc.vector.tensor_copy(
                    out=v_sb[:, tt, oh * 8:(oh + 1) * 8, 0:HD],
                    in_=ps[:].rearrange("p (h d) -> p h d", d=HD),
                )

            # ---- LayerNorm residual; 512x-scaled constants make res_sb =
            # 512*residual so the final evict's 1/512 recovers it exactly.
            # Elementwise work on DVE (GPSIMD's 2-scalar tensor_scalar is a
            # ~13us software path that serialized the whole kernel); x stages
            # in-place in res_sb. Stats for all 8 blocks batch into ONE ACT
            # Sqrt so the activation table switches sqrt<->exp only once. ----
            mvall = lnpool.tile([P, KB, 2], F32, tag="mvall", name="mvall",
                                bufs=1)
            rstdall = lnpool.tile([P, 2, KB], F32, tag="rstdall",
                                  name="rstdall", bufs=1)

            def ln_stats(it):
                x_t = res_sb[:, it, :]
                nc.sync.dma_start(x_t, xq_h[:][it * P:(it + 1) * P, :])
                stats = lnpool.tile([P, 2, 6], F32, tag="stats", name="stats")
                for g in range(2):
                    nc.vector.bn_stats(out=stats[:, g, :],
                                       in_=x_t[:, g * 512:(g + 1) * 512])
                nc.vector.bn_aggr(out=mvall[:, it, :], in_=stats[:])

            def ln_rstd():
                # rstdall row 0 = 1/std, row 1 = mu/std (all 8 blocks at once)
                nc.scalar.activation(out=rstdall[:, 0, :],
                                     in_=mvall[:, :, 1], func=AF.Sqrt,
                                     bias=eps_t[:])
                nc.vector.reciprocal(out=rstdall[:, 0, :],
                                     in_=rstdall[:, 0, :])
                nc.vector.tensor_mul(out=rstdall[:, 1, :],
                                     in0=mvall[:, :, 0],
                                     in1=rstdall[:, 0, :])

            def ln_norm(it):
                x_t = res_sb[:, it, :]
                nc.vector.tensor_scalar(out=x_t, in0=x_t,
                                        scalar1=rstdall[:, 0, it:it + 1],
                                        scalar2=rstdall[:, 1, it:it + 1],
                                        op0=ALU.mult, op1=ALU.subtract)
                nc.vector.tensor_mul(out=x_t, in0=x_t, in1=lnw_b[:])
                nc.vector.tensor_add(out=x_t, in0=x_t, in1=lnbbo_b[:])

            # ---- attention: scores^T -> exp -> mask -> ctx^T -> recips ----
            # 128-query blocks x 3 key blocks of 128 (the middle key block is
            # fully inside the band and needs no mask), so exp/mask/matmul
            # volume is 3/4 of the 256-query variant. Head-PAIR iterations:
            # the even head's score matmuls contract on PE rows 0-63, the odd
            # head's on rows 64-127 (tile_position auto-derived from the lhsT
            # base partition) so the hardware runs each jb's pair
            # concurrently. Software-pipelined with a 2-iteration lookahead
            # so the in-order PE never waits on exp/mask.
            NQ8 = SQ // P      # 8 query blocks
            JB3 = 3            # key blocks per query block
            # q8 = 1, 0 go LAST: they are the only blocks reading the x^T
            # halo columns (keys < 256), whose k-projection chunk streams in
            # the background; pairs stay qb-adjacent so finish_qb still
            # fires every 16 items
            # halo-consuming blocks 1, 0 run EARLY (their k-halo chunk heads
            # the bg queue) so all other bg deadlines relax and the bg work
            # spreads deep into the item stream instead of ending at item ~31
            q8_order = [2, 3, 1, 0, 4, 5, 6, 7]
            items = [(q8, hb) for q8 in q8_order for hb in range(NH // 2)]
            probs_of = {}

            def emit_scores(i):
                q8, hb = items[i]
                probs = work.tile([P, 2, JB3, P], BF16, tag="probs",
                                  name="probs", bufs=3)
                ps_s = spsum.tile([P, 2, 4, P], F32, tag="sc", name="ps_s")
                for jb in range(JB3):
                    j0 = (q8 + jb) * P
                    for par in range(2):
                        ho = par * HD
                        nc.tensor.matmul(
                            ps_s[:, par, jb, :],
                            kT_sb[ho:ho + HD, hb, j0:j0 + P],
                            qT_sb[ho:ho + HD, hb, q8 * P:(q8 + 1) * P],
                            start=True, stop=True,
                        )
                nc.scalar.activation(out=probs[:],
                                     in_=ps_s[:, :, 0:JB3, :], func=AF.Exp)
                # band mask: one affine inequality over (key row r, query
                # col c) per outer key block, on the Pool engine. r =
                # partition, parity is a dead dim (step 0), c is the last
                # free dim. Keep where A >= 0, zero elsewhere:
                #   jb0: r-c-1>=0   jb1: always in band   jb2: c-r>=0
                for jb, (ch, cstep, base) in ((0, (1, -1, -1)),
                                              (2, (-1, 1, 0))):
                    nc.gpsimd.affine_select(
                        out=probs[:, :, jb, :], in_=probs[:, :, jb, :],
                        compare_op=ALU.is_ge, fill=0.0, base=base,
                        pattern=[[0, 2], [cstep, P]],
                        channel_multiplier=ch)
                probs_of[i] = probs

            def emit_ctx(i):
                q8, hb = items[i]
                probs = probs_of.pop(i)
                ps_c = cpsum.tile([HD + 1, 2, P], F32, tag="cx", name="ps_c")
                for par in range(2):
                    for jb in range(JB3):
                        nc.tensor.matmul(
                            ps_c[:, par, :],
                            v_sb[:, q8 + jb, 2 * hb + par, :],
                            probs[:, par, jb, :],
                            start=(jb == 0), stop=(jb == JB3 - 1),
                        )
                qs = slice(q8 * P, (q8 + 1) * P)
                # one CAST evicts both heads' ctx AND the denominator row
                with nc.allow_low_precision(
                        reason="softmax denom in bf16: 0.4% rel "
                               "on a 2e-2 budget"):
                    nc.vector.tensor_copy(out=ct_sb[:, :, hb, qs],
                                          in_=ps_c[:, :, :])

            def finish_a(qlo, qlen):
                # stage 1: gather the denominator row (ct partition 64) to
                # [head, q] rows with one DMA (pure DMA -- no engine waits)
                qs = slice(qlo, qlo + qlen)
                # recip_sb rows are par-major: head h = 2*hb+par at row
                # par*8+hb, so [par, hb] merges into one stride-1024 dim
                den = ct_sb[HD:HD + 1, :, :, qs]
                nc.sync.dma_start(
                    recip_sb[:, qs],
                    bass.AP(tensor=den.tensor, offset=den.offset,
                            ap=[list(den.ap)[0]] +
                               [[KB * SQ, 2], [SQ, KB], [1, qlen]]))

            def finish_b(qlo, qlen):
                # stage 2 (two items later, so the gather DMA has landed and
                # the reciprocal does not stall the DVE queue): reciprocate,
                # bounce through DRAM, and spread each head's row across its
                # 64 ct partitions so the normalize is a plain DVE mult
                qs = slice(qlo, qlo + qlen)
                qb, off = divmod(qlo, QBS)
                with nc.allow_low_precision(
                        reason="softmax denom recip in bf16: 0.4% rel "
                               "on a 2e-2 budget"):
                    nc.vector.reciprocal(out=recip_sb[:, qs],
                                         in_=recip_sb[:, qs])
                nc.sync.dma_start(rs_h[:][qb, :, off:off + qlen],
                                  recip_sb[:, qs])
                rcp2 = work.tile([HD, 2, KB, qlen], BF16, tag="rcp2",
                                 name="rcp2", bufs=1)
                for h in range(NH):
                    src = rs_h[:][qb, (h % 2) * KB + h // 2, off:off + qlen]
                    nc.sync.dma_start(
                        rcp2[:, h % 2, h // 2, :],
                        bass.AP(tensor=src.tensor, offset=src.offset,
                                ap=[[0, HD]] + list(src.ap)))
                rcp2_of[qlo] = rcp2

            # ---- final tasks: normalize ctx^T into fp8 (x8 scale via
            # vones=8) and out-project in fp8 DoubleRow + residual via
            # identity matmul. Both are sliced per 128-query block, so block
            # q8's 10 tasks interleave with block q8+1's attention pairs
            # (PE/Pool have slack there while ACT runs exp); only q8=7's
            # tasks trail the last pair. ----
            ct8 = big.tile([P, KB, SQ], F8, tag="xtr8", name="ct8")

            def emit_norm(qlo, qlen, hb, par):
                # normalize ct by the DMA-spread reciprocals: plain SBUF
                # bf16 DVE multiply (no PE selector matmul needed); par halves
                # relocate [0:64] -> [par*64:...] like the old 2-CAST evict
                qs = slice(qlo, qlo + qlen)
                nc.vector.tensor_mul(out=ct8[par * HD:(par + 1) * HD, hb, qs],
                                     in0=ct_sb[0:HD, par, hb, qs],
                                     in1=rcp2_of[qlo][:, par, hb, :])

            def emit_out_mms(it, oh):
                ps_o = gpsum.tile([P, 512], F32, tag="gp", name="ps_o")
                for j in range(KB2):
                    nc.tensor.matmul(
                        ps_o[:],
                        ct8[:, 2 * j:2 * j + 2, it * P:(it + 1) * P],
                        wo8_sl[j][:, :, oh * 512:(oh + 1) * 512],
                        start=(j == 0), stop=False,
                        perf_mode=DR,
                    )
                nc.tensor.matmul(
                    ps_o[:], ident_sb[:],
                    res_sb[:, it, oh * 512:(oh + 1) * 512],
                    start=False, stop=True,
                )

                def evict():
                    o_t = work.tile([P, 512], F32, tag="o_t", name="o_t",
                                    bufs=2)
                    nc.scalar.mul(o_t[:], ps_o[:], 1.0 / OSC)
                    nc.sync.dma_start(
                        out_h[:][it * P:(it + 1) * P,
                                 oh * 512:(oh + 1) * 512],
                        o_t[:])
                return evict

            def final_tasks(qlo, qlen):
                for hb in range(KB):
                    yield lambda hb=hb: (emit_norm(qlo, qlen, hb, 0),
                                         emit_norm(qlo, qlen, hb, 1))
                # out-proj evictions trail their matmuls by two task slots so
                # the ACT-queue copy never waits on an unfinished PSUM group
                ev = []
                for it in range(qlo // P, (qlo + qlen) // P):
                    for oh in range(2):
                        def mm(it=it, oh=oh):
                            ev.append(emit_out_mms(it, oh))
                        yield mm
                        if len(ev) >= (2 if oh == 1 else 3):
                            yield ev.pop(0)
                while ev:
                    yield ev.pop(0)

            rcp2_of = {}

            # ---- pre-phase: just enough projection work for the first
            # attention items (q chunk0, k chunk0), then all v groups (they
            # are DoubleRow-cheap, and xt8 -- whose SBUF ring slot ct8
            # reuses -- must be fully consumed before the first normalize
            # task writes ct8), then LayerNorm ----
            for ob in range(KB):
                qk_group(qT_sb, wq_sl, ob, WIN, 0, 512)
            for ob in range(KB):
                qk_group(kT_sb, wk_sl, ob, 0, 256, 512)
            for tt in range(SE // P):
                v_group(tt, 0)
                v_group(tt, 1)
            # LayerNorm last in the pre-phase; all stats first, then the one
            # batched Sqrt. The 8 normalize units drip between the first
            # attention items (below) so their ~14us of DVE elementwise work
            # does not delay the first ctx evictions.
            for it in range(KB):
                ln_stats(it)
            ln_rstd()

            # ---- remaining projection groups drip-fed between attention
            # items, ordered by deadline (k halo for blocks 1,0 first, then
            # the k/q chunks for blocks 4-7, ob-interleaved to match the
            # per-head consumption order) ----
            def bgwork():
                for ob in range(KB):
                    yield lambda ob=ob: qk_mms(kT_sb, wk_sl, ob, 0, 0, 256)
                for ob in range(KB):
                    yield lambda ob=ob: qk_mms(kT_sb, wk_sl, ob, 0, 768, 512)
                for ob in range(KB):
                    yield lambda ob=ob: qk_mms(qT_sb, wq_sl, ob, WIN, 512,
                                               512)

            bg = bgwork()
            pending = []
            fb_at = {}
            bg_evict = None
            delay = 0
            emit_scores(0)
            emit_scores(1)
            for i in range(len(items)):
                if i + 2 < len(items):
                    emit_scores(i + 2)
                emit_ctx(i)
                if i < KB:
                    ln_norm(i)
                # halo chunks must be emitted by item ~14 (blocks 1,0 start
                # at 16) and each q512(ob) before the block-4 scores that
                # read it (emitted at item 30+ob). Each group's eviction is
                # deferred to the NEXT item so its PSUM dependency lands
                # before the copy reaches the head of the ACT queue.
                if bg_evict is not None:
                    bg_evict()
                    bg_evict = None
                if i < 16 or i % 2 == 0:
                    bgtask = next(bg, None)
                    if bgtask is not None:
                        bg_evict = bgtask()
                if (i + 1) % NH == 0:
                    fin = ((items[i][0] // 2) * QBS, QBS)
                    finish_a(*fin)
                    fb_at[i + 2] = fin
                if i in fb_at:
                    fin = fb_at.pop(i)
                    finish_b(*fin)
                    pending.append(final_tasks(*fin))
                    # let the spread-DMA latency pass before the first norm
                    # task queues on DVE
                    delay = 2
                if delay > 0:
                    delay -= 1
                else:
                    # 10 tasks per 8-item block -> 2 tasks/item keeps the
                    # out-proj right behind attention
                    for _ in range(2):
                        while pending:
                            task = next(pending[0], None)
                            if task is not None:
                                task()
                                break
                            pending.pop(0)
            if bg_evict is not None:
                bg_evict()
            for fin in fb_at.values():
                finish_b(*fin)
                pending.append(final_tasks(*fin))
            for gen in pending:
                for task in gen:
                    task()


_CACHE = {}


def get_nc():
    if "nc" not in _CACHE:
        _CACHE["nc"] = build_nc()
    return _CACHE["nc"]


def make_in_maps(inputs):
    x = np.asarray(inputs["hidden_states"], dtype=np.float32)
    wq = np.asarray(inputs["wq"], dtype=np.float32)
    wk = np.asarray(inputs["wk"], dtype=np.float32)
    wv = np.asarray(inputs["wv"], dtype=np.float32)
    wo = np.asarray(inputs["wo"], dtype=np.float32)
    bo = np.asarray(inputs["bo"], dtype=np.float32)
    ln_w = np.asarray(inputs["ln_w"], dtype=np.float32)
    ln_b = np.asarray(inputs["ln_b"], dtype=np.float32)

    bf = ml_dtypes.bfloat16
    f8 = ml_dtypes.float8_e4m3
    wqT = np.ascontiguousarray(wq.T).astype(bf)
    wkT = np.ascontiguousarray(wk.T).astype(bf)
    wv8 = np.clip(np.ascontiguousarray(wv.T) * WSC, -240, 240).astype(f8)
    wo8 = np.clip(np.ascontiguousarray(wo.T) * WSC, -240, 240).astype(f8)
    lnw = (ln_w * OSC).astype(bf)
    lnbbo = ((ln_b + bo) * OSC).astype(bf)
    ident = np.eye(P, dtype=np.float32).astype(bf)

    in_maps = []
    for core in range(NCORES):
        b, hh = divmod(core, 2)
        start = hh * SQ
        xkv = np.zeros((SE, H), dtype=np.float32)
        xkv[WIN:] = x[b, start:start + SQ]
        vones = np.full(SE, WSC / CSC, dtype=np.float32)
        if start > 0:
            xkv[:WIN] = x[b, start - WIN:start]
        else:
            vones[:WIN] = 0.0
        xkvT = np.ascontiguousarray(xkv.T)
        in_maps.append({
            "xq": np.ascontiguousarray(x[b, start:start + SQ]).astype(bf),
            "xT": xkvT.astype(bf),
            "xT8": np.clip(xkvT, -240, 240).astype(f8),
            "wqT": wqT, "wkT": wkT, "wv8": wv8, "wo8": wo8,
            "lnw": lnw, "lnbbo": lnbbo,
            "vones": vones.astype(bf),
            "ident": ident,
        })
    return in_maps


def kernel(**inputs):
    from concourse.bass_utils import run_bass_kernel_spmd
    nc = get_nc()
    in_maps = make_in_maps(inputs)
    res = run_bass_kernel_spmd(nc, in_maps, core_ids=list(range(NCORES)))
    out = np.empty((B, S, H), dtype=np.float32)
    for core in range(NCORES):
        b, hh = divmod(core, 2)
        out[b, hh * SQ:(hh + 1) * SQ, :] = res.results[core]["out"]
    return out

